# revision 8
# baseline (speedup 1.0000x reference)
"""Bi-directional cross-attention kernel for Trainium2 (8 NeuronCores).

Problem: x_1, x_2: [8, 2048, 1024] fp32; 6 projection weights [1024, 1024].
  ctx2 = softmax((x1 Wq1)(x2 Wk2)^T / 32) (x2 Wv2)
  ctx1 = softmax((x2 Wq2)(x1 Wk1)^T / 32) (x1 Wv1)
Returns (ctx1, ctx2), each [8, 2048, 1024] fp32.

Sharding: batch dim (8) across the 8 cores — pure data parallel, no
collectives. Each core runs both attention directions for its batch element.

Per-core kernel design (bf16 matmuls, fp32 PSUM accumulation):
- Host feeds x TRANSPOSED (xT [1024, 2048] bf16) so the contraction dim
  (d_in) lands on SBUF partitions for the projections.
- qT, kT are produced in [d, s] layout (lhsT = W slice, rhs = xT slice);
  v in natural [s, d] layout (lhsT = xT slice, rhs = Wv slice).
- Scores are computed TRANSPOSED: S^T[sk, sq] = sum_e kT[e,sk] qT[e,sq],
  so after exp (ScalarE, scale=1/32 folded in) the P^T tiles feed the AV
  matmul directly as the stationary operand — no transposes anywhere.
- softmax skips max-subtraction (scores ~ N(0,1), |s/32| < ~6 — exp is
  safe in fp32/bf16); row sums come from an extra ones-column matmul
  (N=1, ~60-cycle floor) accumulated alongside AV; normalization happens
  on the small ctx output via ScalarE Copy with per-partition scale.
"""

import os

import numpy as np
import ml_dtypes

import concourse.bass as bass
import concourse.tile as tile
from concourse import mybir
from concourse.bass_utils import run_bass_kernel_spmd
from concourse.vector_clock import ScopedClock, VectorClock

BF16 = mybir.dt.bfloat16
F32 = mybir.dt.float32

S = 2048  # sequence length per stream
D = 1024  # d_in == d_kq == d_v
P = 128   # SBUF partitions
NB = 512  # matmul moving-operand free-size / PSUM bank (fp32)
N_CORES = 8
SCALE = 1.0 / 32.0  # 1/sqrt(D_KQ)


def _drain_and_barrier_split(self, tick_clock, wait_clock):
    """Workaround: this walrus build allows at most ONE sync-wait on
    CTRL-class (Drain/Nop) instructions, but Tile's kernel-tail drain
    attaches one wait per outstanding logical processor ("Too many sync
    wait commands"). Split the waits across single-wait NOPs on the sync
    engine (program order makes them cumulative), then drain bare."""
    gc = tick_clock.global_clock
    n = len(gc)
    for i in range(n):
        t = gc[i]
        if t <= 0:
            continue
        vec = [0] * n
        vec[i] = t
        nop = self.nc.sync.nop(nofuse=True, hint=f"drain_wait_p{i}")
        wait_clock.add_sem_waits(nop.ins, ScopedClock({None: VectorClock(vec)}))
        si = nop.ins.sync_info
        nw = len(si.on_wait) if si is not None else 0
        assert nw <= 1, f"proc {i} produced {nw} waits on drain-split nop"
    self.nc.sync.drain()
    self.nc.all_engine_barrier()
    assert self.sems is not None
    popped = self.nc._tile_sem_poison_stack.pop()
    assert popped is self._sem_poison
    self.nc.clear_and_free_semaphores(list(self.sems.allocated().values()))
    self.nc.all_engine_barrier()


tile.TileContext._drain_and_barrier = _drain_and_barrier_split

_NOP_N = [0]


def _split_multi_waits(ordered):
    """Same walrus limitation as above, general case: Tile attaches up to
    3 sync-waits to DMA/compute instructions; this build accepts one.
    Move all but one wait onto fresh single-wait NOPs on the same engine,
    inserted immediately before the instruction (program order on the
    engine makes the waits cumulative)."""
    for insts in ordered.values():
        new = []
        for inst in insts:
            si = inst.sync_info
            waits = list(si.on_wait) if si is not None else []
            if len(waits) > 1:
                assert all(w.wait_reg is None for w in waits), inst.name
                for w in waits[:-1]:
                    _NOP_N[0] += 1
                    nop = mybir.InstNoOp(
                        name=f"I-waitsplit-{_NOP_N[0]}", ins=[], outs=[])
                    nop.engine = inst.engine
                    nop.sync_info = mybir.SyncInfo(on_wait=[w], on_update=[])
                    new.append(nop)
                inst.sync_info = mybir.SyncInfo(
                    on_wait=[waits[-1]], on_update=list(si.on_update))
            new.append(inst)
        insts[:] = new


_ORIG_LOWER = tile.TileContext._lower_ordered_insts


def _lower_patched(self, ordered):
    _split_multi_waits(ordered)
    return _ORIG_LOWER(self, ordered)


tile.TileContext._lower_ordered_insts = _lower_patched


def _direction(nc, pools, xTq, xTkv, w_dram, out_ap, ones, late_loads=()):
    """One cross-attention direction.

    xTq:  list of 8 SBUF tiles [128, S] bf16 — query-side x, transposed
    xTkv: list of 8 SBUF tiles [128, S] bf16 — key/value-side x, transposed
    w_dram: (Wq, Wk, Wv) DRAM APs [D, D] bf16
    out_ap: DRAM AP [S, D] fp32
    late_loads: (dst_sbuf_ap, src_dram_ap) pairs whose DMAs must not race
        the first projection's loads — they are emitted gated on the kT
        projection's progress.
    """
    from concourse.tile_rust import add_dep_helper
    wq_d, wk_d, wv_d = w_dram
    wpool, kTp, vp, qpool, ptpool, ctxpool, rpool, mm, av = pools
    CI = D // P    # contraction chunks over d_in
    M8 = D // P    # d_out tiles
    CK = S // P    # sk chunks
    SQB = S // NB  # sq blocks
    MS = NB // P   # sq subtiles per block
    DVB = D // NB  # dv blocks

    # ---- kT [d_kq, sk]: 8 tiles [128, S] ----
    # The kernel's first matmuls consume Wk + xTkv: stage those DMAs in
    # consumption order (Wk m=0 column, first xTkv column-block, rest of Wk,
    # remaining xTkv blocks) and walk the loop skb-outer so the PE starts
    # within a few microseconds instead of waiting out the initial burst.
    # (Only matters for direction A; for B everything is already resident.)
    wk_t = [wpool.tile([P, D], BF16, tag="w", name=f"wk_{ci}") for ci in range(CI)]
    for ci in range(CI):
        nc.sync.dma_start(wk_t[ci][:, 0:P], wk_d[ci * P:(ci + 1) * P, 0:P])
    for ci in range(CI):
        nc.sync.dma_start(wk_t[ci][:, P:4 * P], wk_d[ci * P:(ci + 1) * P, P:4 * P])
    for ci in range(CI):
        nc.sync.dma_start(wk_t[ci][:, 4 * P:D], wk_d[ci * P:(ci + 1) * P, 4 * P:D])
    kT = [kTp.tile([P, S], BF16, tag="kT", name=f"kT_{m}") for m in range(M8)]
    kT_copies = []
    for skb in range(SQB):
        for m in range(M8):
            ps = mm.tile([P, NB], F32, tag="mm", name="ps")
            for ci in range(CI):
                nc.tensor.matmul(
                    ps[:], wk_t[ci][:, m * P:(m + 1) * P],
                    xTkv[ci][:, skb * NB:(skb + 1) * NB],
                    start=(ci == 0), stop=(ci == CI - 1),
                )
            kT_copies.append(
                nc.vector.tensor_copy(kT[m][:, skb * NB:(skb + 1) * NB], ps[:]))

    # Late loads: gate on kT-projection progress so they don't steal HBM
    # bandwidth from the startup-critical Wk/xTkv transfers.
    for j, (dst, src) in enumerate(late_loads):
        dma = nc.sync.dma_start(dst, src)
        anchor = kT_copies[min(j, len(kT_copies) - 1)]
        add_dep_helper(dma.ins, anchor.ins, reason="late-load gating")

    # ---- v [sk, d_v]: 16 tiles [128, D] ----
    wv_t = [wpool.tile([P, D], BF16, tag="w", name=f"wv_{ci}") for ci in range(CI)]
    for ci in range(CI):
        dma = nc.sync.dma_start(wv_t[ci][:], wv_d[ci * P:(ci + 1) * P, :])
        add_dep_helper(dma.ins, kT_copies[ci].ins, reason="wv prefetch gating")
    v = [vp.tile([P, D], BF16, tag="v", name=f"v_{s}") for s in range(CK)]
    for s16 in range(CK):
        for dvb in range(DVB):
            ps = mm.tile([P, NB], F32, tag="mm", name="ps")
            for ci in range(CI):
                nc.tensor.matmul(
                    ps[:], xTkv[ci][:, s16 * P:(s16 + 1) * P],
                    wv_t[ci][:, dvb * NB:(dvb + 1) * NB],
                    start=(ci == 0), stop=(ci == CI - 1),
                )
            nc.vector.tensor_copy(v[s16][:, dvb * NB:(dvb + 1) * NB], ps[:])

    # ---- per sq-block: qT block, S^T, exp, AV ----
    wq_t = [wpool.tile([P, D], BF16, tag="w", name=f"wq_{ci}") for ci in range(CI)]
    for ci in range(CI):
        nc.sync.dma_start(wq_t[ci][:], wq_d[ci * P:(ci + 1) * P, :])
    for sqb in range(SQB):
        qb = [qpool.tile([P, NB], BF16, tag="qb", name=f"qb_{m}") for m in range(M8)]
        for m in range(M8):
            ps = mm.tile([P, NB], F32, tag="mm", name="ps")
            for ci in range(CI):
                nc.tensor.matmul(
                    ps[:], wq_t[ci][:, m * P:(m + 1) * P],
                    xTq[ci][:, sqb * NB:(sqb + 1) * NB],
                    start=(ci == 0), stop=(ci == CI - 1),
                )
            nc.vector.tensor_copy(qb[m][:], ps[:])

        # S^T[sk-chunk, sq-block] then P^T = exp(S^T / 32)
        pt = [ptpool.tile([P, NB], BF16, tag="pt", name=f"pt_{ck}") for ck in range(CK)]
        for ck in range(CK):
            ps = mm.tile([P, NB], F32, tag="mm", name="ps")
            for m in range(M8):
                nc.tensor.matmul(
                    ps[:], kT[m][:, ck * P:(ck + 1) * P], qb[m][:],
                    start=(m == 0), stop=(m == M8 - 1),
                )
            nc.scalar.activation(
                pt[ck][:], ps[:], mybir.ActivationFunctionType.Exp, scale=SCALE,
            )

        # ctx[sq, dv] + row sums (ones-column matmul in its own 1-bank psum
        # tile), normalize via per-partition scale, store
        for ms in range(MS):
            acc = av.tile([P, 2 * NB], F32, tag="av", name="acc")
            rs = mm.tile([P, 1], F32, tag="mm", name="rs")
            for ck in range(CK):
                lhs = pt[ck][:, ms * P:(ms + 1) * P]
                st, sp = (ck == 0), (ck == CK - 1)
                nc.tensor.matmul(acc[:, 0:NB], lhs, v[ck][:, 0:NB], start=st, stop=sp)
                nc.tensor.matmul(acc[:, NB:2 * NB], lhs, v[ck][:, NB:2 * NB],
                                 start=st, stop=sp)
                nc.tensor.matmul(rs[:], lhs, ones[:], start=st, stop=sp)
            r = rpool.tile([P, 1], F32, tag="r", name="r")
            nc.vector.reciprocal(r[:], rs[:])
            c = ctxpool.tile([P, D], F32, tag="ctx", name="c")
            nc.scalar.activation(
                c[:], acc[:, 0:D], mybir.ActivationFunctionType.Copy, scale=r[:],
            )
            row = (sqb * MS + ms) * P
            nc.sync.dma_start(out_ap[row:row + P, :], c[:])


def build_nc():
    nc = bass.Bass()
    x1T = nc.dram_tensor("x1T", [D, S], BF16, kind="ExternalInput").ap()
    x2T = nc.dram_tensor("x2T", [D, S], BF16, kind="ExternalInput").ap()
    w = {
        name: nc.dram_tensor(name, [D, D], BF16, kind="ExternalInput").ap()
        for name in ("wq1", "wk1", "wv1", "wq2", "wk2", "wv2")
    }
    ctx1 = nc.dram_tensor("ctx1", [S, D], F32, kind="ExternalOutput").ap()
    ctx2 = nc.dram_tensor("ctx2", [S, D], F32, kind="ExternalOutput").ap()

    CI = D // P
    with tile.TileContext(nc) as tc:
        with (
            tc.tile_pool(name="xT", bufs=2 * CI) as xpool,
            tc.tile_pool(name="w", bufs=16) as wpool,
            tc.tile_pool(name="kTp", bufs=CI) as kTp,
            tc.tile_pool(name="vp", bufs=S // P) as vp,
            tc.tile_pool(name="qb", bufs=12) as qpool,
            tc.tile_pool(name="pt", bufs=S // P + 2) as ptpool,
            tc.tile_pool(name="ctx", bufs=3) as ctxpool,
            tc.tile_pool(name="r", bufs=4) as rpool,
            tc.tile_pool(name="misc", bufs=1) as misc,
            tc.tile_pool(name="mm", bufs=4, space=bass.MemorySpace.PSUM) as mm,
            tc.tile_pool(name="av", bufs=2, space=bass.MemorySpace.PSUM) as av,
        ):
            x1T_t = [xpool.tile([P, S], BF16, tag="xT", name=f"x1T_{ci}") for ci in range(CI)]
            x2T_t = [xpool.tile([P, S], BF16, tag="xT", name=f"x2T_{ci}") for ci in range(CI)]
            # Startup-critical loads (x2T feeds the first projection): front
            # half of each tile first, the rest behind it. x1T is not needed
            # until ~110us in — emitted as gated late_loads inside direction A.
            for ci in range(CI):
                nc.sync.dma_start(x2T_t[ci][:, 0:NB], x2T[ci * P:(ci + 1) * P, 0:NB])
            for cb in range(1, S // NB):
                for ci in range(CI):
                    nc.sync.dma_start(
                        x2T_t[ci][:, cb * NB:(cb + 1) * NB],
                        x2T[ci * P:(ci + 1) * P, cb * NB:(cb + 1) * NB])
            ones = misc.tile([P, 1], BF16)
            nc.gpsimd.memset(ones[:], 1.0)

            late = [
                (x1T_t[ci][:], x1T[ci * P:(ci + 1) * P, :]) for ci in range(CI)
            ]
            pools = (wpool, kTp, vp, qpool, ptpool, ctxpool, rpool, mm, av)
            # ctx2: q from x1 (Wq1), k/v from x2 (Wk2, Wv2)
            _direction(nc, pools, x1T_t, x2T_t, (w["wq1"], w["wk2"], w["wv2"]),
                       ctx2, ones, late_loads=late)
            # ctx1: q from x2 (Wq2), k/v from x1 (Wk1, Wv1)
            _direction(nc, pools, x2T_t, x1T_t, (w["wq2"], w["wk1"], w["wv1"]),
                       ctx1, ones)
    return nc


_NC_CACHE = None


def _enable_ntff_tracing():
    """Dev-only (KERNEL_TRACE=1): register the axon NTFF profile hook that
    this image's `antenv` package lacks, and stub out the artifact upload
    (no bucket creds in-container). The graded path never sets KERNEL_TRACE,
    so none of this runs there."""
    import sys
    import types

    if "antenv.axon_hooks" not in sys.modules:
        m = types.ModuleType("antenv.axon_hooks")
        m._hook = None

        def set_axon_ntff_profile_hook(h):
            m._hook = h

        def get_axon_ntff_profile_hook():
            return m._hook

        m.set_axon_ntff_profile_hook = set_axon_ntff_profile_hook
        m.get_axon_ntff_profile_hook = get_axon_ntff_profile_hook
        sys.modules["antenv.axon_hooks"] = m
        import antenv

        antenv.axon_hooks = m
    mod = sys.modules["antenv.axon_hooks"]
    if mod._hook is None:
        from trn_agent_boot.trn_boot import _ntff_profile_via_ctypes

        mod._hook = _ntff_profile_via_ctypes("/opt/axon/libaxon_pjrt.so")
    import concourse.bass_utils as bu

    bu.upload_artifacts = lambda tmpdir: tmpdir


def kernel(x_1, x_2, W_query_1, W_key_1, W_value_1, W_query_2, W_key_2,
           W_value_2):
    global _NC_CACHE
    bf = ml_dtypes.bfloat16
    B = x_1.shape[0]
    assert B == N_CORES and x_1.shape == (B, S, D)

    weights = {
        "wq1": np.asarray(W_query_1, np.float32).astype(bf),
        "wk1": np.asarray(W_key_1, np.float32).astype(bf),
        "wv1": np.asarray(W_value_1, np.float32).astype(bf),
        "wq2": np.asarray(W_query_2, np.float32).astype(bf),
        "wk2": np.asarray(W_key_2, np.float32).astype(bf),
        "wv2": np.asarray(W_value_2, np.float32).astype(bf),
    }
    x_1 = np.asarray(x_1, np.float32)
    x_2 = np.asarray(x_2, np.float32)
    in_maps = [
        {"x1T": x_1[b].T.astype(bf), "x2T": x_2[b].T.astype(bf), **weights}
        for b in range(B)
    ]

    if _NC_CACHE is None:
        _NC_CACHE = build_nc()
    trace = bool(os.environ.get("KERNEL_TRACE"))
    if trace:
        _enable_ntff_tracing()
    res = run_bass_kernel_spmd(_NC_CACHE, in_maps, core_ids=list(range(N_CORES)),
                               trace=trace)
    if trace and res.exec_time_ns is not None:
        print(f"HW exec time: {res.exec_time_ns} ns")
        if res.instructions_and_trace is not None:
            print(f"trace: {res.instructions_and_trace[1]}")
    ctx1 = np.stack([res.results[b]["ctx1"] for b in range(B)])
    ctx2 = np.stack([res.results[b]["ctx2"] for b in range(B)])
    return ctx1, ctx2


# revision 9
# speedup vs baseline: 1.0186x; 1.0186x over previous
"""Bi-directional cross-attention kernel for Trainium2 (8 NeuronCores).

Problem: x_1, x_2: [8, 2048, 1024] fp32; 6 projection weights [1024, 1024].
  ctx2 = softmax((x1 Wq1)(x2 Wk2)^T / 32) (x2 Wv2)
  ctx1 = softmax((x2 Wq2)(x1 Wk1)^T / 32) (x1 Wv1)
Returns (ctx1, ctx2), each [8, 2048, 1024] fp32.

Sharding: batch dim (8) across the 8 cores — pure data parallel, no
collectives. Each core runs both attention directions for its batch element.

Per-core kernel design (bf16 matmuls, fp32 PSUM accumulation):
- Host feeds x TRANSPOSED (xT [1024, 2048] bf16) so the contraction dim
  (d_in) lands on SBUF partitions for the projections.
- qT, kT are produced in [d, s] layout (lhsT = W slice, rhs = xT slice);
  v in natural [s, d] layout (lhsT = xT slice, rhs = Wv slice).
- Scores are computed TRANSPOSED: S^T[sk, sq] = sum_e kT[e,sk] qT[e,sq],
  so after exp (ScalarE, scale=1/32 folded in) the P^T tiles feed the AV
  matmul directly as the stationary operand — no transposes anywhere.
- softmax skips max-subtraction (scores ~ N(0,1), |s/32| < ~6 — exp is
  safe in fp32/bf16); row sums come from an extra ones-column matmul
  (N=1, ~60-cycle floor) accumulated alongside AV; normalization happens
  on the small ctx output via ScalarE Copy with per-partition scale.
"""

import os

import numpy as np
import ml_dtypes

import concourse.bass as bass
import concourse.tile as tile
from concourse import mybir
from concourse.bass_utils import run_bass_kernel_spmd
from concourse.vector_clock import ScopedClock, VectorClock

BF16 = mybir.dt.bfloat16
F32 = mybir.dt.float32

S = 2048  # sequence length per stream
D = 1024  # d_in == d_kq == d_v
P = 128   # SBUF partitions
NB = 512  # matmul moving-operand free-size / PSUM bank (fp32)
N_CORES = 8
SCALE = 1.0 / 32.0  # 1/sqrt(D_KQ)


def _drain_and_barrier_split(self, tick_clock, wait_clock):
    """Workaround: this walrus build allows at most ONE sync-wait on
    CTRL-class (Drain/Nop) instructions, but Tile's kernel-tail drain
    attaches one wait per outstanding logical processor ("Too many sync
    wait commands"). Split the waits across single-wait NOPs on the sync
    engine (program order makes them cumulative), then drain bare."""
    gc = tick_clock.global_clock
    n = len(gc)
    for i in range(n):
        t = gc[i]
        if t <= 0:
            continue
        vec = [0] * n
        vec[i] = t
        nop = self.nc.sync.nop(nofuse=True, hint=f"drain_wait_p{i}")
        wait_clock.add_sem_waits(nop.ins, ScopedClock({None: VectorClock(vec)}))
        si = nop.ins.sync_info
        nw = len(si.on_wait) if si is not None else 0
        assert nw <= 1, f"proc {i} produced {nw} waits on drain-split nop"
    self.nc.sync.drain()
    self.nc.all_engine_barrier()
    assert self.sems is not None
    popped = self.nc._tile_sem_poison_stack.pop()
    assert popped is self._sem_poison
    self.nc.clear_and_free_semaphores(list(self.sems.allocated().values()))
    self.nc.all_engine_barrier()


tile.TileContext._drain_and_barrier = _drain_and_barrier_split

_NOP_N = [0]


def _split_multi_waits(ordered):
    """Same walrus limitation as above, general case: Tile attaches up to
    3 sync-waits to DMA/compute instructions; this build accepts one.
    Move all but one wait onto fresh single-wait NOPs on the same engine,
    inserted immediately before the instruction (program order on the
    engine makes the waits cumulative)."""
    for insts in ordered.values():
        new = []
        for inst in insts:
            si = inst.sync_info
            waits = list(si.on_wait) if si is not None else []
            if len(waits) > 1:
                assert all(w.wait_reg is None for w in waits), inst.name
                for w in waits[:-1]:
                    _NOP_N[0] += 1
                    nop = mybir.InstNoOp(
                        name=f"I-waitsplit-{_NOP_N[0]}", ins=[], outs=[])
                    nop.engine = inst.engine
                    nop.sync_info = mybir.SyncInfo(on_wait=[w], on_update=[])
                    new.append(nop)
                inst.sync_info = mybir.SyncInfo(
                    on_wait=[waits[-1]], on_update=list(si.on_update))
            new.append(inst)
        insts[:] = new


_ORIG_LOWER = tile.TileContext._lower_ordered_insts


def _lower_patched(self, ordered):
    _split_multi_waits(ordered)
    return _ORIG_LOWER(self, ordered)


tile.TileContext._lower_ordered_insts = _lower_patched


def _direction(nc, pools, xTq, xTkv, w_dram, out_ap, ones, late_loads=(),
               gate_dmas=()):
    """One cross-attention direction.

    xTq:  list of 8 SBUF tiles [128, S] bf16 — query-side x, transposed
    xTkv: list of 8 SBUF tiles [128, S] bf16 — key/value-side x, transposed
    w_dram: (Wq, Wk, Wv) DRAM APs [D, D] bf16
    out_ap: DRAM AP [S, D] fp32
    late_loads: (dst_sbuf_ap, src_dram_ap) pairs whose DMAs must not race
        the first projection's loads — they are emitted gated on the kT
        projection's progress.
    """
    from concourse.tile_rust import add_dep_helper
    wq_d, wk_d, wv_d = w_dram
    wpool, kTp, vp, qpool, ptpool, ctxpool, rpool, mm, av = pools
    CI = D // P    # contraction chunks over d_in
    M8 = D // P    # d_out tiles
    CK = S // P    # sk chunks
    SQB = S // NB  # sq blocks
    MS = NB // P   # sq subtiles per block
    DVB = D // NB  # dv blocks

    # ---- kT [d_kq, sk]: 8 tiles [128, S] ----
    # The kernel's first matmuls consume Wk + xTkv: stage those DMAs in
    # consumption order (Wk m=0 column, first xTkv column-block, rest of Wk,
    # remaining xTkv blocks) and walk the loop skb-outer so the PE starts
    # within a few microseconds instead of waiting out the initial burst.
    # (Only matters for direction A; for B everything is already resident.)
    wk_t = [wpool.tile([P, D], BF16, tag="w", name=f"wk_{ci}") for ci in range(CI)]
    for ci in range(CI):
        nc.sync.dma_start(wk_t[ci][:], wk_d[ci * P:(ci + 1) * P, :])
    kT = [kTp.tile([P, S], BF16, tag="kT", name=f"kT_{m}") for m in range(M8)]
    kT_copies = []
    for skb in range(SQB):
        for m in range(M8):
            ps = mm.tile([P, NB], F32, tag="mm", name="ps")
            for ci in range(CI):
                nc.tensor.matmul(
                    ps[:], wk_t[ci][:, m * P:(m + 1) * P],
                    xTkv[ci][:, skb * NB:(skb + 1) * NB],
                    start=(ci == 0), stop=(ci == CI - 1),
                )
            kT_copies.append(
                nc.vector.tensor_copy(kT[m][:, skb * NB:(skb + 1) * NB], ps[:]))

    # xTkv tail column-blocks: block cb is first consumed by skb=cb groups —
    # gate each on a copy from the previous block so its transfer runs during,
    # not before, the preceding block's compute.
    for j, dma in enumerate(gate_dmas):
        skb_needed = 1 + j // CI
        anchor = kT_copies[(skb_needed - 1) * M8 + (j % CI)]
        add_dep_helper(dma.ins, anchor.ins, reason="xT tail gating")

    # Late loads: gate on kT-projection progress so they don't steal HBM
    # bandwidth from the startup-critical Wk/xTkv transfers.
    for j, (dst, src) in enumerate(late_loads):
        dma = nc.sync.dma_start(dst, src)
        anchor = kT_copies[min(3 * M8 + j, len(kT_copies) - 1)]
        add_dep_helper(dma.ins, anchor.ins, reason="late-load gating")

    # ---- v [sk, d_v]: 16 tiles [128, D] ----
    wv_t = [wpool.tile([P, D], BF16, tag="w", name=f"wv_{ci}") for ci in range(CI)]
    for ci in range(CI):
        dma = nc.sync.dma_start(wv_t[ci][:], wv_d[ci * P:(ci + 1) * P, :])
        add_dep_helper(dma.ins, kT_copies[M8 + ci].ins, reason="wv prefetch gating")
    v = [vp.tile([P, D], BF16, tag="v", name=f"v_{s}") for s in range(CK)]
    for s16 in range(CK):
        for dvb in range(DVB):
            ps = mm.tile([P, NB], F32, tag="mm", name="ps")
            for ci in range(CI):
                nc.tensor.matmul(
                    ps[:], xTkv[ci][:, s16 * P:(s16 + 1) * P],
                    wv_t[ci][:, dvb * NB:(dvb + 1) * NB],
                    start=(ci == 0), stop=(ci == CI - 1),
                )
            nc.vector.tensor_copy(v[s16][:, dvb * NB:(dvb + 1) * NB], ps[:])

    # ---- per sq-block: qT block, S^T, exp, AV ----
    wq_t = [wpool.tile([P, D], BF16, tag="w", name=f"wq_{ci}") for ci in range(CI)]
    for ci in range(CI):
        nc.sync.dma_start(wq_t[ci][:], wq_d[ci * P:(ci + 1) * P, :])
    for sqb in range(SQB):
        qb = [qpool.tile([P, NB], BF16, tag="qb", name=f"qb_{m}") for m in range(M8)]
        for m in range(M8):
            ps = mm.tile([P, NB], F32, tag="mm", name="ps")
            for ci in range(CI):
                nc.tensor.matmul(
                    ps[:], wq_t[ci][:, m * P:(m + 1) * P],
                    xTq[ci][:, sqb * NB:(sqb + 1) * NB],
                    start=(ci == 0), stop=(ci == CI - 1),
                )
            nc.vector.tensor_copy(qb[m][:], ps[:])

        # S^T[sk-chunk, sq-block] then P^T = exp(S^T / 32)
        pt = [ptpool.tile([P, NB], BF16, tag="pt", name=f"pt_{ck}") for ck in range(CK)]
        for ck in range(CK):
            ps = mm.tile([P, NB], F32, tag="mm", name="ps")
            for m in range(M8):
                nc.tensor.matmul(
                    ps[:], kT[m][:, ck * P:(ck + 1) * P], qb[m][:],
                    start=(m == 0), stop=(m == M8 - 1),
                )
            nc.scalar.activation(
                pt[ck][:], ps[:], mybir.ActivationFunctionType.Exp, scale=SCALE,
            )

        # ctx[sq, dv] + row sums (ones-column matmul in its own 1-bank psum
        # tile), normalize via per-partition scale, store
        for ms in range(MS):
            acc = av.tile([P, 2 * NB], F32, tag="av", name="acc")
            rs = mm.tile([P, 1], F32, tag="mm", name="rs")
            for ck in range(CK):
                lhs = pt[ck][:, ms * P:(ms + 1) * P]
                st, sp = (ck == 0), (ck == CK - 1)
                nc.tensor.matmul(acc[:, 0:NB], lhs, v[ck][:, 0:NB], start=st, stop=sp)
                nc.tensor.matmul(acc[:, NB:2 * NB], lhs, v[ck][:, NB:2 * NB],
                                 start=st, stop=sp)
                nc.tensor.matmul(rs[:], lhs, ones[:], start=st, stop=sp)
            r = rpool.tile([P, 1], F32, tag="r", name="r")
            nc.vector.reciprocal(r[:], rs[:])
            c = ctxpool.tile([P, D], F32, tag="ctx", name="c")
            nc.scalar.activation(
                c[:], acc[:, 0:D], mybir.ActivationFunctionType.Copy, scale=r[:],
            )
            row = (sqb * MS + ms) * P
            nc.sync.dma_start(out_ap[row:row + P, :], c[:])


def build_nc():
    nc = bass.Bass()
    x1T = nc.dram_tensor("x1T", [D, S], BF16, kind="ExternalInput").ap()
    x2T = nc.dram_tensor("x2T", [D, S], BF16, kind="ExternalInput").ap()
    w = {
        name: nc.dram_tensor(name, [D, D], BF16, kind="ExternalInput").ap()
        for name in ("wq1", "wk1", "wv1", "wq2", "wk2", "wv2")
    }
    ctx1 = nc.dram_tensor("ctx1", [S, D], F32, kind="ExternalOutput").ap()
    ctx2 = nc.dram_tensor("ctx2", [S, D], F32, kind="ExternalOutput").ap()

    CI = D // P
    with tile.TileContext(nc) as tc:
        with (
            tc.tile_pool(name="xT", bufs=2 * CI) as xpool,
            tc.tile_pool(name="w", bufs=16) as wpool,
            tc.tile_pool(name="kTp", bufs=CI) as kTp,
            tc.tile_pool(name="vp", bufs=S // P) as vp,
            tc.tile_pool(name="qb", bufs=12) as qpool,
            tc.tile_pool(name="pt", bufs=S // P + 2) as ptpool,
            tc.tile_pool(name="ctx", bufs=3) as ctxpool,
            tc.tile_pool(name="r", bufs=4) as rpool,
            tc.tile_pool(name="misc", bufs=1) as misc,
            tc.tile_pool(name="mm", bufs=4, space=bass.MemorySpace.PSUM) as mm,
            tc.tile_pool(name="av", bufs=2, space=bass.MemorySpace.PSUM) as av,
        ):
            x1T_t = [xpool.tile([P, S], BF16, tag="xT", name=f"x1T_{ci}") for ci in range(CI)]
            x2T_t = [xpool.tile([P, S], BF16, tag="xT", name=f"x2T_{ci}") for ci in range(CI)]
            # Startup-critical loads (x2T feeds the first projection): front
            # half of each tile first, the rest behind it. x1T is not needed
            # until ~110us in — emitted as gated late_loads inside direction A.
            for ci in range(CI):
                nc.sync.dma_start(x2T_t[ci][:, 0:NB], x2T[ci * P:(ci + 1) * P, 0:NB])
            x2T_tail_dmas = []
            for cb in range(1, S // NB):
                for ci in range(CI):
                    x2T_tail_dmas.append(nc.sync.dma_start(
                        x2T_t[ci][:, cb * NB:(cb + 1) * NB],
                        x2T[ci * P:(ci + 1) * P, cb * NB:(cb + 1) * NB]))
            ones = misc.tile([P, 1], BF16)
            nc.gpsimd.memset(ones[:], 1.0)

            late = [
                (x1T_t[ci][:], x1T[ci * P:(ci + 1) * P, :]) for ci in range(CI)
            ]
            pools = (wpool, kTp, vp, qpool, ptpool, ctxpool, rpool, mm, av)
            # ctx2: q from x1 (Wq1), k/v from x2 (Wk2, Wv2)
            _direction(nc, pools, x1T_t, x2T_t, (w["wq1"], w["wk2"], w["wv2"]),
                       ctx2, ones, late_loads=late, gate_dmas=x2T_tail_dmas)
            # ctx1: q from x2 (Wq2), k/v from x1 (Wk1, Wv1)
            _direction(nc, pools, x2T_t, x1T_t, (w["wq2"], w["wk1"], w["wv1"]),
                       ctx1, ones)
    return nc


_NC_CACHE = None


def _enable_ntff_tracing():
    """Dev-only (KERNEL_TRACE=1): register the axon NTFF profile hook that
    this image's `antenv` package lacks, and stub out the artifact upload
    (no bucket creds in-container). The graded path never sets KERNEL_TRACE,
    so none of this runs there."""
    import sys
    import types

    if "antenv.axon_hooks" not in sys.modules:
        m = types.ModuleType("antenv.axon_hooks")
        m._hook = None

        def set_axon_ntff_profile_hook(h):
            m._hook = h

        def get_axon_ntff_profile_hook():
            return m._hook

        m.set_axon_ntff_profile_hook = set_axon_ntff_profile_hook
        m.get_axon_ntff_profile_hook = get_axon_ntff_profile_hook
        sys.modules["antenv.axon_hooks"] = m
        import antenv

        antenv.axon_hooks = m
    mod = sys.modules["antenv.axon_hooks"]
    if mod._hook is None:
        from trn_agent_boot.trn_boot import _ntff_profile_via_ctypes

        mod._hook = _ntff_profile_via_ctypes("/opt/axon/libaxon_pjrt.so")
    import concourse.bass_utils as bu

    bu.upload_artifacts = lambda tmpdir: tmpdir


def kernel(x_1, x_2, W_query_1, W_key_1, W_value_1, W_query_2, W_key_2,
           W_value_2):
    global _NC_CACHE
    bf = ml_dtypes.bfloat16
    B = x_1.shape[0]
    assert B == N_CORES and x_1.shape == (B, S, D)

    weights = {
        "wq1": np.asarray(W_query_1, np.float32).astype(bf),
        "wk1": np.asarray(W_key_1, np.float32).astype(bf),
        "wv1": np.asarray(W_value_1, np.float32).astype(bf),
        "wq2": np.asarray(W_query_2, np.float32).astype(bf),
        "wk2": np.asarray(W_key_2, np.float32).astype(bf),
        "wv2": np.asarray(W_value_2, np.float32).astype(bf),
    }
    x_1 = np.asarray(x_1, np.float32)
    x_2 = np.asarray(x_2, np.float32)
    in_maps = [
        {"x1T": x_1[b].T.astype(bf), "x2T": x_2[b].T.astype(bf), **weights}
        for b in range(B)
    ]

    if _NC_CACHE is None:
        _NC_CACHE = build_nc()
    trace = bool(os.environ.get("KERNEL_TRACE"))
    if trace:
        _enable_ntff_tracing()
    res = run_bass_kernel_spmd(_NC_CACHE, in_maps, core_ids=list(range(N_CORES)),
                               trace=trace)
    if trace and res.exec_time_ns is not None:
        print(f"HW exec time: {res.exec_time_ns} ns")
        if res.instructions_and_trace is not None:
            print(f"trace: {res.instructions_and_trace[1]}")
    ctx1 = np.stack([res.results[b]["ctx1"] for b in range(B)])
    ctx2 = np.stack([res.results[b]["ctx2"] for b in range(B)])
    return ctx1, ctx2


# revision 10
# speedup vs baseline: 1.0187x; 1.0002x over previous
"""Bi-directional cross-attention kernel for Trainium2 (8 NeuronCores).

Problem: x_1, x_2: [8, 2048, 1024] fp32; 6 projection weights [1024, 1024].
  ctx2 = softmax((x1 Wq1)(x2 Wk2)^T / 32) (x2 Wv2)
  ctx1 = softmax((x2 Wq2)(x1 Wk1)^T / 32) (x1 Wv1)
Returns (ctx1, ctx2), each [8, 2048, 1024] fp32.

Sharding: batch dim (8) across the 8 cores — pure data parallel, no
collectives. Each core runs both attention directions for its batch element.

Per-core kernel design (bf16 matmuls, fp32 PSUM accumulation):
- Host feeds x TRANSPOSED (xT [1024, 2048] bf16) so the contraction dim
  (d_in) lands on SBUF partitions for the projections.
- qT, kT are produced in [d, s] layout (lhsT = W slice, rhs = xT slice);
  v in natural [s, d] layout (lhsT = xT slice, rhs = Wv slice).
- Scores are computed TRANSPOSED: S^T[sk, sq] = sum_e kT[e,sk] qT[e,sq],
  so after exp (ScalarE, scale=1/32 folded in) the P^T tiles feed the AV
  matmul directly as the stationary operand — no transposes anywhere.
- softmax skips max-subtraction (scores ~ N(0,1), |s/32| < ~6 — exp is
  safe in fp32/bf16); row sums come from an extra ones-column matmul
  (N=1, ~60-cycle floor) accumulated alongside AV; normalization happens
  on the small ctx output via ScalarE Copy with per-partition scale.
"""

import os

import numpy as np
import ml_dtypes

import concourse.bass as bass
import concourse.tile as tile
from concourse import mybir
from concourse.bass_utils import run_bass_kernel_spmd
from concourse.vector_clock import ScopedClock, VectorClock

BF16 = mybir.dt.bfloat16
F32 = mybir.dt.float32

S = 2048  # sequence length per stream
D = 1024  # d_in == d_kq == d_v
P = 128   # SBUF partitions
NB = 512  # matmul moving-operand free-size / PSUM bank (fp32)
N_CORES = 8
SCALE = 1.0 / 32.0  # 1/sqrt(D_KQ)


def _drain_and_barrier_split(self, tick_clock, wait_clock):
    """Workaround: this walrus build allows at most ONE sync-wait on
    CTRL-class (Drain/Nop) instructions, but Tile's kernel-tail drain
    attaches one wait per outstanding logical processor ("Too many sync
    wait commands"). Split the waits across single-wait NOPs on the sync
    engine (program order makes them cumulative), then drain bare."""
    gc = tick_clock.global_clock
    n = len(gc)
    for i in range(n):
        t = gc[i]
        if t <= 0:
            continue
        vec = [0] * n
        vec[i] = t
        nop = self.nc.sync.nop(nofuse=True, hint=f"drain_wait_p{i}")
        wait_clock.add_sem_waits(nop.ins, ScopedClock({None: VectorClock(vec)}))
        si = nop.ins.sync_info
        nw = len(si.on_wait) if si is not None else 0
        assert nw <= 1, f"proc {i} produced {nw} waits on drain-split nop"
    self.nc.sync.drain()
    self.nc.all_engine_barrier()
    assert self.sems is not None
    popped = self.nc._tile_sem_poison_stack.pop()
    assert popped is self._sem_poison
    self.nc.clear_and_free_semaphores(list(self.sems.allocated().values()))
    self.nc.all_engine_barrier()


tile.TileContext._drain_and_barrier = _drain_and_barrier_split

_NOP_N = [0]


def _split_multi_waits(ordered):
    """Same walrus limitation as above, general case: Tile attaches up to
    3 sync-waits to DMA/compute instructions; this build accepts one.
    Move all but one wait onto fresh single-wait NOPs on the same engine,
    inserted immediately before the instruction (program order on the
    engine makes the waits cumulative)."""
    for insts in ordered.values():
        new = []
        for inst in insts:
            si = inst.sync_info
            waits = list(si.on_wait) if si is not None else []
            if len(waits) > 1:
                assert all(w.wait_reg is None for w in waits), inst.name
                for w in waits[:-1]:
                    _NOP_N[0] += 1
                    nop = mybir.InstNoOp(
                        name=f"I-waitsplit-{_NOP_N[0]}", ins=[], outs=[])
                    nop.engine = inst.engine
                    nop.sync_info = mybir.SyncInfo(on_wait=[w], on_update=[])
                    new.append(nop)
                inst.sync_info = mybir.SyncInfo(
                    on_wait=[waits[-1]], on_update=list(si.on_update))
            new.append(inst)
        insts[:] = new


_ORIG_LOWER = tile.TileContext._lower_ordered_insts


def _lower_patched(self, ordered):
    _split_multi_waits(ordered)
    return _ORIG_LOWER(self, ordered)


tile.TileContext._lower_ordered_insts = _lower_patched


def _copy(nc, idx, dst, src_ps):
    """Projection psum->sbuf copies, alternated between DVE and the (otherwise
    idle during projections) ScalarE so neither engine serializes the drain."""
    if idx % 2 == 0:
        return nc.vector.tensor_copy(dst, src_ps)
    return nc.scalar.activation(dst, src_ps, mybir.ActivationFunctionType.Copy)


def _direction(nc, pools, xTq, xTkv, w_dram, out_ap, ones, late_loads=(),
               gate_dmas=()):
    """One cross-attention direction.

    xTq:  list of 8 SBUF tiles [128, S] bf16 — query-side x, transposed
    xTkv: list of 8 SBUF tiles [128, S] bf16 — key/value-side x, transposed
    w_dram: (Wq, Wk, Wv) DRAM APs [D, D] bf16
    out_ap: DRAM AP [S, D] fp32
    late_loads: (dst_sbuf_ap, src_dram_ap) pairs whose DMAs must not race
        the first projection's loads — they are emitted gated on the kT
        projection's progress.
    """
    from concourse.tile_rust import add_dep_helper
    wq_d, wk_d, wv_d = w_dram
    wpool, kTp, vp, qpool, ptpool, ctxpool, rpool, mm, av = pools
    CI = D // P    # contraction chunks over d_in
    M8 = D // P    # d_out tiles
    CK = S // P    # sk chunks
    SQB = S // NB  # sq blocks
    MS = NB // P   # sq subtiles per block
    DVB = D // NB  # dv blocks

    # ---- kT [d_kq, sk]: 8 tiles [128, S] ----
    # The kernel's first matmuls consume Wk + xTkv: stage those DMAs in
    # consumption order (Wk m=0 column, first xTkv column-block, rest of Wk,
    # remaining xTkv blocks) and walk the loop skb-outer so the PE starts
    # within a few microseconds instead of waiting out the initial burst.
    # (Only matters for direction A; for B everything is already resident.)
    wk_t = [wpool.tile([P, D], BF16, tag="w", name=f"wk_{ci}") for ci in range(CI)]
    for ci in range(CI):
        nc.sync.dma_start(wk_t[ci][:], wk_d[ci * P:(ci + 1) * P, :])
    kT = [kTp.tile([P, S], BF16, tag="kT", name=f"kT_{m}") for m in range(M8)]
    kT_copies = []
    for skb in range(SQB):
        for m in range(M8):
            ps = mm.tile([P, NB], F32, tag="mm", name="ps")
            for ci in range(CI):
                nc.tensor.matmul(
                    ps[:], wk_t[ci][:, m * P:(m + 1) * P],
                    xTkv[ci][:, skb * NB:(skb + 1) * NB],
                    start=(ci == 0), stop=(ci == CI - 1),
                )
            kT_copies.append(
                _copy(nc, skb * M8 + m, kT[m][:, skb * NB:(skb + 1) * NB], ps[:]))

    # xTkv tail column-blocks: block cb is first consumed by skb=cb groups —
    # gate each on a copy from the previous block so its transfer runs during,
    # not before, the preceding block's compute.
    for j, dma in enumerate(gate_dmas):
        skb_needed = 1 + j // CI
        anchor = kT_copies[(skb_needed - 1) * M8 + (j % CI)]
        add_dep_helper(dma.ins, anchor.ins, reason="xT tail gating")

    # Late loads: gate on kT-projection progress so they don't steal HBM
    # bandwidth from the startup-critical Wk/xTkv transfers.
    for j, (dst, src) in enumerate(late_loads):
        dma = nc.sync.dma_start(dst, src)
        anchor = kT_copies[min(3 * M8 + j, len(kT_copies) - 1)]
        add_dep_helper(dma.ins, anchor.ins, reason="late-load gating")

    # ---- v [sk, d_v]: 16 tiles [128, D] ----
    wv_t = [wpool.tile([P, D], BF16, tag="w", name=f"wv_{ci}") for ci in range(CI)]
    for ci in range(CI):
        dma = nc.sync.dma_start(wv_t[ci][:], wv_d[ci * P:(ci + 1) * P, :])
        add_dep_helper(dma.ins, kT_copies[M8 + ci].ins, reason="wv prefetch gating")
    v = [vp.tile([P, D], BF16, tag="v", name=f"v_{s}") for s in range(CK)]
    for s16 in range(CK):
        for dvb in range(DVB):
            ps = mm.tile([P, NB], F32, tag="mm", name="ps")
            for ci in range(CI):
                nc.tensor.matmul(
                    ps[:], xTkv[ci][:, s16 * P:(s16 + 1) * P],
                    wv_t[ci][:, dvb * NB:(dvb + 1) * NB],
                    start=(ci == 0), stop=(ci == CI - 1),
                )
            _copy(nc, s16 * DVB + dvb, v[s16][:, dvb * NB:(dvb + 1) * NB], ps[:])

    # ---- per sq-block: qT block, S^T, exp, AV ----
    wq_t = [wpool.tile([P, D], BF16, tag="w", name=f"wq_{ci}") for ci in range(CI)]
    for ci in range(CI):
        nc.sync.dma_start(wq_t[ci][:], wq_d[ci * P:(ci + 1) * P, :])
    for sqb in range(SQB):
        qb = [qpool.tile([P, NB], BF16, tag="qb", name=f"qb_{m}") for m in range(M8)]
        for m in range(M8):
            ps = mm.tile([P, NB], F32, tag="mm", name="ps")
            for ci in range(CI):
                nc.tensor.matmul(
                    ps[:], wq_t[ci][:, m * P:(m + 1) * P],
                    xTq[ci][:, sqb * NB:(sqb + 1) * NB],
                    start=(ci == 0), stop=(ci == CI - 1),
                )
            _copy(nc, m, qb[m][:], ps[:])

        # S^T[sk-chunk, sq-block] then P^T = exp(S^T / 32)
        pt = [ptpool.tile([P, NB], BF16, tag="pt", name=f"pt_{ck}") for ck in range(CK)]
        for ck in range(CK):
            ps = mm.tile([P, NB], F32, tag="mm", name="ps")
            for m in range(M8):
                nc.tensor.matmul(
                    ps[:], kT[m][:, ck * P:(ck + 1) * P], qb[m][:],
                    start=(m == 0), stop=(m == M8 - 1),
                )
            nc.scalar.activation(
                pt[ck][:], ps[:], mybir.ActivationFunctionType.Exp, scale=SCALE,
            )

        # ctx[sq, dv] + row sums (ones-column matmul in its own 1-bank psum
        # tile), normalize via per-partition scale, store
        for ms in range(MS):
            acc = av.tile([P, 2 * NB], F32, tag="av", name="acc")
            rs = mm.tile([P, 1], F32, tag="mm", name="rs")
            for ck in range(CK):
                lhs = pt[ck][:, ms * P:(ms + 1) * P]
                st, sp = (ck == 0), (ck == CK - 1)
                nc.tensor.matmul(acc[:, 0:NB], lhs, v[ck][:, 0:NB], start=st, stop=sp)
                nc.tensor.matmul(acc[:, NB:2 * NB], lhs, v[ck][:, NB:2 * NB],
                                 start=st, stop=sp)
                nc.tensor.matmul(rs[:], lhs, ones[:], start=st, stop=sp)
            r = rpool.tile([P, 1], F32, tag="r", name="r")
            nc.vector.reciprocal(r[:], rs[:])
            c = ctxpool.tile([P, D], F32, tag="ctx", name="c")
            row = (sqb * MS + ms) * P
            for h in range(2):
                nc.scalar.activation(
                    c[:, h * NB:(h + 1) * NB], acc[:, h * NB:(h + 1) * NB],
                    mybir.ActivationFunctionType.Copy, scale=r[:],
                )
                nc.sync.dma_start(out_ap[row:row + P, h * NB:(h + 1) * NB],
                                  c[:, h * NB:(h + 1) * NB])


def build_nc():
    nc = bass.Bass()
    x1T = nc.dram_tensor("x1T", [D, S], BF16, kind="ExternalInput").ap()
    x2T = nc.dram_tensor("x2T", [D, S], BF16, kind="ExternalInput").ap()
    w = {
        name: nc.dram_tensor(name, [D, D], BF16, kind="ExternalInput").ap()
        for name in ("wq1", "wk1", "wv1", "wq2", "wk2", "wv2")
    }
    ctx1 = nc.dram_tensor("ctx1", [S, D], F32, kind="ExternalOutput").ap()
    ctx2 = nc.dram_tensor("ctx2", [S, D], F32, kind="ExternalOutput").ap()

    CI = D // P
    with tile.TileContext(nc) as tc:
        with (
            tc.tile_pool(name="xT", bufs=2 * CI) as xpool,
            tc.tile_pool(name="w", bufs=16) as wpool,
            tc.tile_pool(name="kTp", bufs=CI) as kTp,
            tc.tile_pool(name="vp", bufs=S // P) as vp,
            tc.tile_pool(name="qb", bufs=12) as qpool,
            tc.tile_pool(name="pt", bufs=S // P + 2) as ptpool,
            tc.tile_pool(name="ctx", bufs=3) as ctxpool,
            tc.tile_pool(name="r", bufs=4) as rpool,
            tc.tile_pool(name="misc", bufs=1) as misc,
            tc.tile_pool(name="mm", bufs=4, space=bass.MemorySpace.PSUM) as mm,
            tc.tile_pool(name="av", bufs=2, space=bass.MemorySpace.PSUM) as av,
        ):
            x1T_t = [xpool.tile([P, S], BF16, tag="xT", name=f"x1T_{ci}") for ci in range(CI)]
            x2T_t = [xpool.tile([P, S], BF16, tag="xT", name=f"x2T_{ci}") for ci in range(CI)]
            # Startup-critical loads (x2T feeds the first projection): front
            # half of each tile first, the rest behind it. x1T is not needed
            # until ~110us in — emitted as gated late_loads inside direction A.
            for ci in range(CI):
                nc.sync.dma_start(x2T_t[ci][:, 0:NB], x2T[ci * P:(ci + 1) * P, 0:NB])
            x2T_tail_dmas = []
            for cb in range(1, S // NB):
                for ci in range(CI):
                    x2T_tail_dmas.append(nc.sync.dma_start(
                        x2T_t[ci][:, cb * NB:(cb + 1) * NB],
                        x2T[ci * P:(ci + 1) * P, cb * NB:(cb + 1) * NB]))
            ones = misc.tile([P, 1], BF16)
            nc.gpsimd.memset(ones[:], 1.0)

            late = [
                (x1T_t[ci][:], x1T[ci * P:(ci + 1) * P, :]) for ci in range(CI)
            ]
            pools = (wpool, kTp, vp, qpool, ptpool, ctxpool, rpool, mm, av)
            # ctx2: q from x1 (Wq1), k/v from x2 (Wk2, Wv2)
            _direction(nc, pools, x1T_t, x2T_t, (w["wq1"], w["wk2"], w["wv2"]),
                       ctx2, ones, late_loads=late, gate_dmas=x2T_tail_dmas)
            # ctx1: q from x2 (Wq2), k/v from x1 (Wk1, Wv1)
            _direction(nc, pools, x2T_t, x1T_t, (w["wq2"], w["wk1"], w["wv1"]),
                       ctx1, ones)
    return nc


_NC_CACHE = None


def _enable_ntff_tracing():
    """Dev-only (KERNEL_TRACE=1): register the axon NTFF profile hook that
    this image's `antenv` package lacks, and stub out the artifact upload
    (no bucket creds in-container). The graded path never sets KERNEL_TRACE,
    so none of this runs there."""
    import sys
    import types

    if "antenv.axon_hooks" not in sys.modules:
        m = types.ModuleType("antenv.axon_hooks")
        m._hook = None

        def set_axon_ntff_profile_hook(h):
            m._hook = h

        def get_axon_ntff_profile_hook():
            return m._hook

        m.set_axon_ntff_profile_hook = set_axon_ntff_profile_hook
        m.get_axon_ntff_profile_hook = get_axon_ntff_profile_hook
        sys.modules["antenv.axon_hooks"] = m
        import antenv

        antenv.axon_hooks = m
    mod = sys.modules["antenv.axon_hooks"]
    if mod._hook is None:
        from trn_agent_boot.trn_boot import _ntff_profile_via_ctypes

        mod._hook = _ntff_profile_via_ctypes("/opt/axon/libaxon_pjrt.so")
    import concourse.bass_utils as bu

    bu.upload_artifacts = lambda tmpdir: tmpdir


def kernel(x_1, x_2, W_query_1, W_key_1, W_value_1, W_query_2, W_key_2,
           W_value_2):
    global _NC_CACHE
    bf = ml_dtypes.bfloat16
    B = x_1.shape[0]
    assert B == N_CORES and x_1.shape == (B, S, D)

    weights = {
        "wq1": np.asarray(W_query_1, np.float32).astype(bf),
        "wk1": np.asarray(W_key_1, np.float32).astype(bf),
        "wv1": np.asarray(W_value_1, np.float32).astype(bf),
        "wq2": np.asarray(W_query_2, np.float32).astype(bf),
        "wk2": np.asarray(W_key_2, np.float32).astype(bf),
        "wv2": np.asarray(W_value_2, np.float32).astype(bf),
    }
    x_1 = np.asarray(x_1, np.float32)
    x_2 = np.asarray(x_2, np.float32)
    in_maps = [
        {"x1T": x_1[b].T.astype(bf), "x2T": x_2[b].T.astype(bf), **weights}
        for b in range(B)
    ]

    if _NC_CACHE is None:
        _NC_CACHE = build_nc()
    trace = bool(os.environ.get("KERNEL_TRACE"))
    if trace:
        _enable_ntff_tracing()
    res = run_bass_kernel_spmd(_NC_CACHE, in_maps, core_ids=list(range(N_CORES)),
                               trace=trace)
    if trace and res.exec_time_ns is not None:
        print(f"HW exec time: {res.exec_time_ns} ns")
        if res.instructions_and_trace is not None:
            print(f"trace: {res.instructions_and_trace[1]}")
    ctx1 = np.stack([res.results[b]["ctx1"] for b in range(B)])
    ctx2 = np.stack([res.results[b]["ctx2"] for b in range(B)])
    return ctx1, ctx2


# revision 11
# speedup vs baseline: 1.0234x; 1.0046x over previous
"""Bi-directional cross-attention kernel for Trainium2 (8 NeuronCores).

Problem: x_1, x_2: [8, 2048, 1024] fp32; 6 projection weights [1024, 1024].
  ctx2 = softmax((x1 Wq1)(x2 Wk2)^T / 32) (x2 Wv2)
  ctx1 = softmax((x2 Wq2)(x1 Wk1)^T / 32) (x1 Wv1)
Returns (ctx1, ctx2), each [8, 2048, 1024] fp32.

Sharding: batch dim (8) across the 8 cores — pure data parallel, no
collectives. Each core runs both attention directions for its batch element.

Per-core kernel design (bf16 matmuls, fp32 PSUM accumulation):
- Host feeds x TRANSPOSED (xT [1024, 2048] bf16) so the contraction dim
  (d_in) lands on SBUF partitions for the projections.
- qT, kT are produced in [d, s] layout (lhsT = W slice, rhs = xT slice);
  v in natural [s, d] layout (lhsT = xT slice, rhs = Wv slice).
- Scores are computed TRANSPOSED: S^T[sk, sq] = sum_e kT[e,sk] qT[e,sq],
  so after exp (ScalarE, scale=1/32 folded in) the P^T tiles feed the AV
  matmul directly as the stationary operand — no transposes anywhere.
- softmax skips max-subtraction (scores ~ N(0,1), |s/32| < ~6 — exp is
  safe in fp32/bf16); row sums come from an extra ones-column matmul
  (N=1, ~60-cycle floor) accumulated alongside AV; normalization happens
  on the small ctx output via ScalarE Copy with per-partition scale.
"""

import os

import numpy as np
import ml_dtypes

import concourse.bass as bass
import concourse.tile as tile
from concourse import mybir
from concourse.bass_utils import run_bass_kernel_spmd
from concourse.vector_clock import ScopedClock, VectorClock

BF16 = mybir.dt.bfloat16
F32 = mybir.dt.float32

S = 2048  # sequence length per stream
D = 1024  # d_in == d_kq == d_v
P = 128   # SBUF partitions
NB = 512  # matmul moving-operand free-size / PSUM bank (fp32)
N_CORES = 8
SCALE = 1.0 / 32.0  # 1/sqrt(D_KQ)


def _drain_and_barrier_split(self, tick_clock, wait_clock):
    """Workaround: this walrus build allows at most ONE sync-wait on
    CTRL-class (Drain/Nop) instructions, but Tile's kernel-tail drain
    attaches one wait per outstanding logical processor ("Too many sync
    wait commands"). Split the waits across single-wait NOPs on the sync
    engine (program order makes them cumulative), then drain bare."""
    gc = tick_clock.global_clock
    n = len(gc)
    for i in range(n):
        t = gc[i]
        if t <= 0:
            continue
        vec = [0] * n
        vec[i] = t
        nop = self.nc.sync.nop(nofuse=True, hint=f"drain_wait_p{i}")
        wait_clock.add_sem_waits(nop.ins, ScopedClock({None: VectorClock(vec)}))
        si = nop.ins.sync_info
        nw = len(si.on_wait) if si is not None else 0
        assert nw <= 1, f"proc {i} produced {nw} waits on drain-split nop"
    self.nc.sync.drain()
    self.nc.all_engine_barrier()
    assert self.sems is not None
    popped = self.nc._tile_sem_poison_stack.pop()
    assert popped is self._sem_poison
    self.nc.clear_and_free_semaphores(list(self.sems.allocated().values()))
    self.nc.all_engine_barrier()


tile.TileContext._drain_and_barrier = _drain_and_barrier_split

_NOP_N = [0]


def _split_multi_waits(ordered):
    """Same walrus limitation as above, general case: Tile attaches up to
    3 sync-waits to DMA/compute instructions; this build accepts one.
    Move all but one wait onto fresh single-wait NOPs on the same engine,
    inserted immediately before the instruction (program order on the
    engine makes the waits cumulative)."""
    for insts in ordered.values():
        new = []
        for inst in insts:
            si = inst.sync_info
            waits = list(si.on_wait) if si is not None else []
            if len(waits) > 1:
                assert all(w.wait_reg is None for w in waits), inst.name
                for w in waits[:-1]:
                    _NOP_N[0] += 1
                    nop = mybir.InstNoOp(
                        name=f"I-waitsplit-{_NOP_N[0]}", ins=[], outs=[])
                    nop.engine = inst.engine
                    nop.sync_info = mybir.SyncInfo(on_wait=[w], on_update=[])
                    new.append(nop)
                inst.sync_info = mybir.SyncInfo(
                    on_wait=[waits[-1]], on_update=list(si.on_update))
            new.append(inst)
        insts[:] = new


_ORIG_LOWER = tile.TileContext._lower_ordered_insts


def _lower_patched(self, ordered):
    _split_multi_waits(ordered)
    return _ORIG_LOWER(self, ordered)


tile.TileContext._lower_ordered_insts = _lower_patched


def _copy(nc, idx, dst, src_ps):
    """Projection psum->sbuf copies, alternated between DVE and the (otherwise
    idle during projections) ScalarE so neither engine serializes the drain."""
    if idx % 2 == 0:
        return nc.vector.tensor_copy(dst, src_ps)
    return nc.scalar.activation(dst, src_ps, mybir.ActivationFunctionType.Copy)


def _direction(nc, pools, xTq, xTkv, w_dram, out_ap, ones, late_loads=(),
               gate_dmas=()):
    """One cross-attention direction.

    xTq:  list of 8 SBUF tiles [128, S] bf16 — query-side x, transposed
    xTkv: list of 8 SBUF tiles [128, S] bf16 — key/value-side x, transposed
    w_dram: (Wq, Wk, Wv) DRAM APs [D, D] bf16
    out_ap: DRAM AP [S, D] fp32
    late_loads: (dst_sbuf_ap, src_dram_ap) pairs whose DMAs must not race
        the first projection's loads — they are emitted gated on the kT
        projection's progress.
    """
    from concourse.tile_rust import add_dep_helper
    wq_d, wk_d, wv_d = w_dram
    wpool, kTp, vp, qpool, ptpool, ctxpool, rpool, mm, av = pools
    CI = D // P    # contraction chunks over d_in
    M8 = D // P    # d_out tiles
    CK = S // P    # sk chunks
    SQB = S // NB  # sq blocks
    MS = NB // P   # sq subtiles per block
    DVB = D // NB  # dv blocks

    # ---- kT [d_kq, sk]: 8 tiles [128, S] ----
    # The kernel's first matmuls consume Wk + xTkv: stage those DMAs in
    # consumption order (Wk m=0 column, first xTkv column-block, rest of Wk,
    # remaining xTkv blocks) and walk the loop skb-outer so the PE starts
    # within a few microseconds instead of waiting out the initial burst.
    # (Only matters for direction A; for B everything is already resident.)
    wk_t = [wpool.tile([P, D], BF16, tag="w", name=f"wk_{ci}") for ci in range(CI)]
    for ci in range(CI):
        nc.sync.dma_start(wk_t[ci][:], wk_d[ci * P:(ci + 1) * P, :])
    kT = [kTp.tile([P, S], BF16, tag="kT", name=f"kT_{m}") for m in range(M8)]
    kT_copies = []
    for skb in range(SQB):
        for m in range(M8):
            ps = mm.tile([P, NB], F32, tag="mm", name="ps")
            for ci in range(CI):
                nc.tensor.matmul(
                    ps[:], wk_t[ci][:, m * P:(m + 1) * P],
                    xTkv[ci][:, skb * NB:(skb + 1) * NB],
                    start=(ci == 0), stop=(ci == CI - 1),
                )
            kT_copies.append(
                _copy(nc, skb * M8 + m, kT[m][:, skb * NB:(skb + 1) * NB], ps[:]))

    # xTkv tail column-blocks: block cb is first consumed by skb=cb groups —
    # gate each on a copy from the previous block so its transfer runs during,
    # not before, the preceding block's compute.
    for j, dma in enumerate(gate_dmas):
        skb_needed = 1 + j // CI
        anchor = kT_copies[(skb_needed - 1) * M8 + (j % CI)]
        add_dep_helper(dma.ins, anchor.ins, reason="xT tail gating")

    # Late loads: gate on kT-projection progress so they don't steal HBM
    # bandwidth from the startup-critical Wk/xTkv transfers.
    for j, (dst, src) in enumerate(late_loads):
        dma = nc.sync.dma_start(dst, src)
        anchor = kT_copies[min(3 * M8 + j, len(kT_copies) - 1)]
        add_dep_helper(dma.ins, anchor.ins, reason="late-load gating")

    # ---- v [sk, d_v]: 16 tiles [128, D] ----
    wv_t = [wpool.tile([P, D], BF16, tag="w", name=f"wv_{ci}") for ci in range(CI)]
    for ci in range(CI):
        dma = nc.sync.dma_start(wv_t[ci][:], wv_d[ci * P:(ci + 1) * P, :])
        add_dep_helper(dma.ins, kT_copies[M8 + ci].ins, reason="wv prefetch gating")
    v = [vp.tile([P, D], BF16, tag="v", name=f"v_{s}") for s in range(CK)]
    for s16 in range(CK):
        for dvb in range(DVB):
            ps = mm.tile([P, NB], F32, tag="mm", name="ps")
            for ci in range(CI):
                nc.tensor.matmul(
                    ps[:], xTkv[ci][:, s16 * P:(s16 + 1) * P],
                    wv_t[ci][:, dvb * NB:(dvb + 1) * NB],
                    start=(ci == 0), stop=(ci == CI - 1),
                )
            _copy(nc, s16 * DVB + dvb, v[s16][:, dvb * NB:(dvb + 1) * NB], ps[:])

    # ---- per sq-block: qT block, S^T, exp, AV ----
    wq_t = [wpool.tile([P, D], BF16, tag="w", name=f"wq_{ci}") for ci in range(CI)]
    for ci in range(CI):
        nc.sync.dma_start(wq_t[ci][:], wq_d[ci * P:(ci + 1) * P, :])
    for sqb in range(SQB):
        qb = [qpool.tile([P, NB], BF16, tag="qb", name=f"qb_{m}") for m in range(M8)]
        for m in range(M8):
            ps = mm.tile([P, NB], F32, tag="mm", name="ps")
            for ci in range(CI):
                nc.tensor.matmul(
                    ps[:], wq_t[ci][:, m * P:(m + 1) * P],
                    xTq[ci][:, sqb * NB:(sqb + 1) * NB],
                    start=(ci == 0), stop=(ci == CI - 1),
                )
            _copy(nc, m, qb[m][:], ps[:])

        # S^T[sk-chunk, sq-block] then P^T = exp(S^T / 32)
        pt = [ptpool.tile([P, NB], BF16, tag="pt", name=f"pt_{ck}") for ck in range(CK)]
        for ck in range(CK):
            ps = mm.tile([P, NB], F32, tag="mm", name="ps")
            for m in range(M8):
                nc.tensor.matmul(
                    ps[:], kT[m][:, ck * P:(ck + 1) * P], qb[m][:],
                    start=(m == 0), stop=(m == M8 - 1),
                )
            nc.scalar.activation(
                pt[ck][:], ps[:], mybir.ActivationFunctionType.Exp, scale=SCALE,
            )

        # ctx[sq, dv] + row sums (ones-column matmul in its own 1-bank psum
        # tile), normalize via per-partition scale, store
        for ms in range(MS):
            acc = av.tile([P, 2 * NB], F32, tag="av", name="acc")
            rs = mm.tile([P, 1], F32, tag="mm", name="rs")
            for ck in range(CK):
                lhs = pt[ck][:, ms * P:(ms + 1) * P]
                st, sp = (ck == 0), (ck == CK - 1)
                nc.tensor.matmul(acc[:, 0:NB], lhs, v[ck][:, 0:NB], start=st, stop=sp)
                nc.tensor.matmul(acc[:, NB:2 * NB], lhs, v[ck][:, NB:2 * NB],
                                 start=st, stop=sp)
                nc.tensor.matmul(rs[:], lhs, ones[:], start=st, stop=sp)
            r = rpool.tile([P, 1], F32, tag="r", name="r")
            nc.vector.reciprocal(r[:], rs[:])
            c = ctxpool.tile([P, D], F32, tag="ctx", name="c")
            row = (sqb * MS + ms) * P
            for h in range(2):
                nc.scalar.activation(
                    c[:, h * NB:(h + 1) * NB], acc[:, h * NB:(h + 1) * NB],
                    mybir.ActivationFunctionType.Copy, scale=r[:],
                )
                nc.sync.dma_start(out_ap[row:row + P, h * NB:(h + 1) * NB],
                                  c[:, h * NB:(h + 1) * NB])


def build_nc():
    nc = bass.Bass()
    x1T = nc.dram_tensor("x1T", [D, S], BF16, kind="ExternalInput").ap()
    x2T = nc.dram_tensor("x2T", [D, S], BF16, kind="ExternalInput").ap()
    w = {
        name: nc.dram_tensor(name, [D, D], BF16, kind="ExternalInput").ap()
        for name in ("wq1", "wk1", "wv1", "wq2", "wk2", "wv2")
    }
    ctx1 = nc.dram_tensor("ctx1", [S, D], F32, kind="ExternalOutput").ap()
    ctx2 = nc.dram_tensor("ctx2", [S, D], F32, kind="ExternalOutput").ap()

    CI = D // P
    with tile.TileContext(nc) as tc:
        with (
            tc.tile_pool(name="xT", bufs=2 * CI) as xpool,
            tc.tile_pool(name="w", bufs=16) as wpool,
            tc.tile_pool(name="kTp", bufs=CI) as kTp,
            tc.tile_pool(name="vp", bufs=S // P) as vp,
            tc.tile_pool(name="qb", bufs=12) as qpool,
            tc.tile_pool(name="pt", bufs=S // P + 2) as ptpool,
            tc.tile_pool(name="ctx", bufs=3) as ctxpool,
            tc.tile_pool(name="r", bufs=4) as rpool,
            tc.tile_pool(name="misc", bufs=1) as misc,
            tc.tile_pool(name="mm", bufs=4, space=bass.MemorySpace.PSUM) as mm,
            tc.tile_pool(name="av", bufs=2, space=bass.MemorySpace.PSUM) as av,
        ):
            x1T_t = [xpool.tile([P, S], BF16, tag="xT", name=f"x1T_{ci}") for ci in range(CI)]
            x2T_t = [xpool.tile([P, S], BF16, tag="xT", name=f"x2T_{ci}") for ci in range(CI)]
            # Startup-critical loads (x2T feeds the first projection): front
            # half of each tile first, the rest behind it. x1T is not needed
            # until ~110us in — emitted as gated late_loads inside direction A.
            for ci in range(CI):
                nc.sync.dma_start(x2T_t[ci][:, 0:NB], x2T[ci * P:(ci + 1) * P, 0:NB])
            x2T_tail_dmas = []
            for cb in range(1, S // NB):
                for ci in range(CI):
                    x2T_tail_dmas.append(nc.sync.dma_start(
                        x2T_t[ci][:, cb * NB:(cb + 1) * NB],
                        x2T[ci * P:(ci + 1) * P, cb * NB:(cb + 1) * NB]))
            ones = misc.tile([P, 1], BF16)
            nc.gpsimd.memset(ones[:], 1.0)

            # PE warmup: ~12 matmuls on scratch data, issued while the first
            # DMAs are in flight. The PE's HAM clock-gate only releases
            # (1.2 -> 2.4 GHz) after ~3.4us of sustained matmul activity;
            # without this, everything up to ~24us runs at half clock.
            warm_in = misc.tile([P, NB], BF16, name="warm_in")
            nc.gpsimd.memset(warm_in[:], 0.0)
            warm_ps = av.tile([P, 2 * NB], F32, tag="av", name="warm_ps")
            for wi in range(12):
                nc.tensor.matmul(warm_ps[:, 0:NB], warm_in[:, 0:P],
                                 warm_in[:], start=True, stop=True)
            warm_out = rpool.tile([P, 1], F32, tag="r", name="warm_out")
            nc.vector.tensor_copy(warm_out[:], warm_ps[:, 0:1])

            late = [
                (x1T_t[ci][:], x1T[ci * P:(ci + 1) * P, :]) for ci in range(CI)
            ]
            pools = (wpool, kTp, vp, qpool, ptpool, ctxpool, rpool, mm, av)
            # ctx2: q from x1 (Wq1), k/v from x2 (Wk2, Wv2)
            _direction(nc, pools, x1T_t, x2T_t, (w["wq1"], w["wk2"], w["wv2"]),
                       ctx2, ones, late_loads=late, gate_dmas=x2T_tail_dmas)
            # ctx1: q from x2 (Wq2), k/v from x1 (Wk1, Wv1)
            _direction(nc, pools, x2T_t, x1T_t, (w["wq2"], w["wk1"], w["wv1"]),
                       ctx1, ones)
    return nc


_NC_CACHE = None


def _enable_ntff_tracing():
    """Dev-only (KERNEL_TRACE=1): register the axon NTFF profile hook that
    this image's `antenv` package lacks, and stub out the artifact upload
    (no bucket creds in-container). The graded path never sets KERNEL_TRACE,
    so none of this runs there."""
    import sys
    import types

    if "antenv.axon_hooks" not in sys.modules:
        m = types.ModuleType("antenv.axon_hooks")
        m._hook = None

        def set_axon_ntff_profile_hook(h):
            m._hook = h

        def get_axon_ntff_profile_hook():
            return m._hook

        m.set_axon_ntff_profile_hook = set_axon_ntff_profile_hook
        m.get_axon_ntff_profile_hook = get_axon_ntff_profile_hook
        sys.modules["antenv.axon_hooks"] = m
        import antenv

        antenv.axon_hooks = m
    mod = sys.modules["antenv.axon_hooks"]
    if mod._hook is None:
        from trn_agent_boot.trn_boot import _ntff_profile_via_ctypes

        mod._hook = _ntff_profile_via_ctypes("/opt/axon/libaxon_pjrt.so")
    import concourse.bass_utils as bu

    bu.upload_artifacts = lambda tmpdir: tmpdir


def kernel(x_1, x_2, W_query_1, W_key_1, W_value_1, W_query_2, W_key_2,
           W_value_2):
    global _NC_CACHE
    bf = ml_dtypes.bfloat16
    B = x_1.shape[0]
    assert B == N_CORES and x_1.shape == (B, S, D)

    weights = {
        "wq1": np.asarray(W_query_1, np.float32).astype(bf),
        "wk1": np.asarray(W_key_1, np.float32).astype(bf),
        "wv1": np.asarray(W_value_1, np.float32).astype(bf),
        "wq2": np.asarray(W_query_2, np.float32).astype(bf),
        "wk2": np.asarray(W_key_2, np.float32).astype(bf),
        "wv2": np.asarray(W_value_2, np.float32).astype(bf),
    }
    x_1 = np.asarray(x_1, np.float32)
    x_2 = np.asarray(x_2, np.float32)
    in_maps = [
        {"x1T": x_1[b].T.astype(bf), "x2T": x_2[b].T.astype(bf), **weights}
        for b in range(B)
    ]

    if _NC_CACHE is None:
        _NC_CACHE = build_nc()
    trace = bool(os.environ.get("KERNEL_TRACE"))
    if trace:
        _enable_ntff_tracing()
    res = run_bass_kernel_spmd(_NC_CACHE, in_maps, core_ids=list(range(N_CORES)),
                               trace=trace)
    if trace and res.exec_time_ns is not None:
        print(f"HW exec time: {res.exec_time_ns} ns")
        if res.instructions_and_trace is not None:
            print(f"trace: {res.instructions_and_trace[1]}")
    ctx1 = np.stack([res.results[b]["ctx1"] for b in range(B)])
    ctx2 = np.stack([res.results[b]["ctx2"] for b in range(B)])
    return ctx1, ctx2
